# revision 12
# baseline (speedup 1.0000x reference)
"""Causal self-attention on 8 TRN2 NeuronCores.

Sharding: core c -> (batch b = c//2, head-group g = c%2); host sums the two
head-group partial yT outputs per batch.

Design (vs v1 baseline):
- All matmul inputs bf16 (host-cast); PSUM accumulation fp32.
- Region-major attention: queries split into four 512-col regions; per
  (pair, region) a jc-sweep accumulates S^T-layout attention into a 1-bank
  [65, 512] PSUM tile per head (65th row = softmax denominator via a ones
  column in v). Fine 128-quantized causal blocking: for key block jc only
  queries >= 128*jc are computed (ragged first chunk, no masking memsets).
- Both heads of a pair share one [128, 1024] score PSUM tile (par0 cols
  0-511, par1 512-1023) so a single ACT exp op serves both heads.
- Softmax normalize: DVE reciprocal on the denominator row,
  GPSIMD partition_broadcast, one DVE multiply -> bf16 aT.
- Out-proj for query region tt is emitted right after the 4 pairs of that
  region, overlapping the next region's attention on PE.
"""
import numpy as np

B, T, D = 4, 2048, 1024
NH_LOCAL = 8
HD = 64
CL = 512
P = 128
CC = D // P
TC = T // P
NPAIR = 4
NREG = 4

_CACHE = {}

CFG = {"st_bufs": 2, "aT_bufs": 3, "py_bufs": 1, "pt_bufs": 4, "pq_bufs": 3,
       "py_from_st": True}


def _emit_qk_pairs(nc, tc, mybir, r, pairs, wqT_r, wkT_r, xT_sb,
                   qT_sb, kT_sb, p1w, p1q, w_prefetch):
    f32 = mybir.dt.float32
    bf16 = mybir.dt.bfloat16
    for p_i in pairs:
        for wi, (w_r, dst, wtag) in enumerate(
                ((wqT_r, qT_sb, "wq"), (wkT_r, kT_sb, "wk"))):
            key = (p_i, wi)
            if key in w_prefetch:
                w_sl = w_prefetch.pop(key)
            else:
                w_sl = p1w.tile([P, CC, P], bf16, tag=f"{wtag}{p_i % 2}")
                nc.sync.dma_start(w_sl[:], w_r[:, :, p_i * P:(p_i + 1) * P])
            for s5 in range(4):
                pq = p1q.tile([P, 512], f32, tag="pq")
                for cc in range(CC):
                    nc.tensor.matmul(
                        pq[:],
                        w_sl[:, cc, :],
                        xT_sb[:, cc, s5 * 512:(s5 + 1) * 512],
                        start=(cc == 0), stop=(cc == CC - 1))
                nc.vector.tensor_copy(
                    dst[:, p_i, s5 * 512:(s5 + 1) * 512], pq[:])


def _emit_attention_region(nc, tc, mybir, r, p_i, rg, qT_sb, kT_sb, v_sb,
                           tri_sb, ones_sb, aT_sb, p2, p2pt, p2aps, p2sps,
                           p3ps):
    f32 = mybir.dt.float32
    bf16 = mybir.dt.bfloat16
    EXP = mybir.ActivationFunctionType.Exp
    MULT = mybir.AluOpType.mult
    r0, r1 = 512 * rg, 512 * (rg + 1)
    aTs = [p2aps.tile([HD + 1, 512], f32, tag="aT",
                      name=f"aT{e}_{p_i}_{rg}_{r}") for e in range(2)]
    njc = r1 // P
    for jc in range(njc):
        cs = max(r0, P * jc)
        c0 = cs - r0            # offset within region (0..384)
        clen = 512 - c0
        st = p2sps.tile([P, 2, 512], f32, tag="st")
        for par in range(2):
            prow = HD * par
            nc.tensor.matmul(
                st[:, par, c0:512],
                kT_sb[prow:prow + HD, p_i, jc * P:(jc + 1) * P],
                qT_sb[prow:prow + HD, p_i, cs:r1],
                start=True, stop=True)
        pt = p2pt.tile([P, 2, 512], bf16, tag="pt")
        nc.scalar.activation(
            pt[:, :, c0:512], st[:, :, c0:512], EXP, scale=0.125)
        if cs == P * jc:  # diagonal block: causal mask
            for par in range(2):
                nc.vector.tensor_tensor(
                    pt[:, par, c0:c0 + P], pt[:, par, c0:c0 + P],
                    tri_sb[:], MULT)
        for par in range(2):
            h = 2 * p_i + par
            nc.tensor.matmul(
                aTs[par][:, c0:512],
                v_sb[:, jc, h, :],
                pt[:, par, c0:512],
                start=(jc == 0), stop=(jc == njc - 1))
    # normalize both heads: 1/denom (row 64) broadcast to 64 partitions via
    # a K=1 PE matmul against a ones column, then one DVE multiply.
    for par in range(2):
        aT_ps = aTs[par]
        rr = p2.tile([P, 512], bf16, tag="rr")
        with nc.allow_low_precision(reason="softmax denom recip"):
            nc.vector.reciprocal(rr[HD:HD + 1, :], aT_ps[HD:HD + 1, :])
        bc = p3ps.tile([P, 512], f32, tag="py")
        nc.tensor.matmul(bc[0:HD, :], ones_sb[HD:HD + 1, 0:HD],
                         rr[HD:HD + 1, :], start=True, stop=True)
        rb = p2.tile([HD, 512], bf16, tag="rb")
        nc.vector.tensor_copy(rb[:], bc[0:HD, :])
        with nc.allow_low_precision(reason="bf16 attention out"):
            if par == 0:
                nc.vector.tensor_tensor(
                    aT_sb[0:HD, p_i, r0:r1], aT_ps[0:HD, :], rb[:], MULT)
            else:
                t64 = p2.tile([HD, 512], bf16, tag="t64")
                nc.vector.tensor_tensor(t64[:], aT_ps[0:HD, :], rb[:], MULT)
                nc.sync.dma_start(aT_sb[HD:P, p_i, r0:r1], t64[:])


def _emit_out_proj_tt(nc, tc, mybir, r, tt, yT_r, aT_sb, wo_sb, p3, p3ps,
                      py_tag="py"):
    f32 = mybir.dt.float32
    for fc in range(CC):
        py = p3ps.tile([P, 512], f32, tag=py_tag)
        for cc in range(NPAIR):
            nc.tensor.matmul(
                py[:],
                wo_sb[:, cc, fc * P:(fc + 1) * P],
                aT_sb[:, cc, tt * 512:(tt + 1) * 512],
                start=(cc == 0), stop=(cc == NPAIR - 1))
        yst = p3.tile([P, 512], f32, tag="yst")
        nc.vector.tensor_copy(yst[:], py[:])
        eng = nc.sync if fc % 2 == 0 else nc.gpsimd
        eng.dma_start(yT_r[:, fc, tt * 512:(tt + 1) * 512], yst[:])


def _build(repeats=1):
    import concourse.bacc as bacc
    import concourse.mybir as mybir
    import concourse.tile as tile
    from contextlib import ExitStack

    f32 = mybir.dt.float32
    bf16 = mybir.dt.bfloat16

    nc = bacc.Bacc("TRN2", target_bir_lowering=False, debug=False)

    xT = nc.dram_tensor("xT", (D, T), bf16, kind="ExternalInput")
    wqT = nc.dram_tensor("wqT", (D, CL), bf16, kind="ExternalInput")
    wkT = nc.dram_tensor("wkT", (D, CL), bf16, kind="ExternalInput")
    wvT = nc.dram_tensor("wvT", (D, CL), bf16, kind="ExternalInput")
    woT = nc.dram_tensor("woT", (CL, D), bf16, kind="ExternalInput")
    tri = nc.dram_tensor("tri", (P, P), bf16, kind="ExternalInput")
    yT = nc.dram_tensor("yT", (D, T), f32, kind="ExternalOutput")

    xT_r = xT.ap().rearrange("(o p) t -> p o t", p=P)
    wqT_r = wqT.ap().rearrange("(o p) f -> p o f", p=P)
    wkT_r = wkT.ap().rearrange("(o p) f -> p o f", p=P)
    wvT_r = wvT.ap().rearrange("(o p) f -> p o f", p=P)
    woT_r = woT.ap().rearrange("(o p) f -> p o f", p=P)
    yT_r = yT.ap().rearrange("(o p) t -> p o t", p=P)

    with tile.TileContext(nc) as tc, ExitStack() as outer:
        persist = outer.enter_context(tc.tile_pool(name="persist", bufs=1))
        tri_sb = persist.tile([P, P], bf16, tag="tri")
        nc.sync.dma_start(tri_sb[:], tri.ap())
        ones_sb = persist.tile([P, HD], bf16, tag="ones")
        nc.vector.memset(ones_sb[:], 1.0)

        for r in range(repeats):
            with tc.tile_pool(name=f"qkv{r}", bufs=1) as qkvp, \
                 tc.tile_pool(name=f"p1x{r}", bufs=1) as p1x, \
                 tc.tile_pool(name=f"p1w{r}", bufs=2) as p1w, \
                 tc.tile_pool(name=f"wo{r}", bufs=1) as wop, \
                 tc.tile_pool(name=f"aTp{r}", bufs=1) as aTp:
                qT_sb = qkvp.tile([P, NPAIR, T], bf16, tag="qT")
                kT_sb = qkvp.tile([P, NPAIR, T], bf16, tag="kT")
                v_sb = qkvp.tile([P, TC, NH_LOCAL, HD + 1], bf16, tag="v")
                nc.vector.memset(v_sb[:, :, :, HD:HD + 1], 1.0)
                xT_sb = p1x.tile([P, CC, T], bf16, tag="xT")
                aT_sb = aTp.tile([P, NPAIR, T], bf16, tag="aT")
                wo_sb = wop.tile([P, NPAIR, D], bf16, tag="wo")

                # pair-0 q,k weight slices first, then xT in t-quarters
                # (all cc of an early t-range land before later t-ranges so
                # the first q/k matmul group unblocks ASAP), then pair-1
                # weights behind the xT stream.
                w_prefetch = {}
                for wi, (w_r, wtag) in enumerate(
                        ((wqT_r, "wq"), (wkT_r, "wk"))):
                    w_sl = p1w.tile([P, CC, P], bf16, tag=f"{wtag}0")
                    nc.sync.dma_start(w_sl[:], w_r[:, :, 0:P])
                    w_prefetch[(0, wi)] = w_sl
                for tq in range(4):
                    for cc in range(CC):
                        nc.sync.dma_start(
                            xT_sb[:, cc, tq * 512:(tq + 1) * 512],
                            xT_r[:, cc, tq * 512:(tq + 1) * 512])
                for wi, (w_r, wtag) in enumerate(
                        ((wqT_r, "wq"), (wkT_r, "wk"))):
                    w_sl = p1w.tile([P, CC, P], bf16, tag=f"{wtag}1")
                    nc.sync.dma_start(w_sl[:], w_r[:, :, P:2 * P])
                    w_prefetch[(1, wi)] = w_sl

                with tc.tile_pool(name=f"p1q{r}", bufs=CFG["pq_bufs"],
                                  space="PSUM") as p1q:
                    # q/k for pair 0 first so attention can start early
                    _emit_qk_pairs(nc, tc, mybir, r, [0], wqT_r, wkT_r,
                                   xT_sb, qT_sb, kT_sb, p1w, p1q, w_prefetch)

                    # v (wv DMA overlaps q/k compute)
                    with tc.tile_pool(name=f"p1wv{r}", bufs=1) as p1wv, \
                         tc.tile_pool(name=f"p1ps{r}", bufs=2,
                                      space="PSUM") as p1ps:
                        wv_sb = p1wv.tile([P, CC, CL], bf16, tag="wv")
                        for cc in range(CC):
                            nc.gpsimd.dma_start(wv_sb[:, cc, :],
                                                wvT_r[:, cc, :])
                        for t_c in range(TC):
                            pv = p1ps.tile([P, CL], f32, tag="pv")
                            for cc in range(CC):
                                nc.tensor.matmul(
                                    pv[:],
                                    xT_sb[:, cc, t_c * P:(t_c + 1) * P],
                                    wv_sb[:, cc, :],
                                    start=(cc == 0), stop=(cc == CC - 1))
                            nc.vector.tensor_copy(
                                v_sb[:, t_c, :, 0:HD],
                                pv[:].rearrange("p (h d) -> p h d",
                                                h=NH_LOCAL))

                    # remaining pairs' q/k; wo load rides along
                    nc.sync.dma_start(wo_sb[:], woT_r)
                    _emit_qk_pairs(nc, tc, mybir, r, [1, 2, 3], wqT_r,
                                   wkT_r, xT_sb, qT_sb, kT_sb, p1w, p1q,
                                   w_prefetch)

                # attention, region-major, with out-proj interleaved
                with tc.tile_pool(name=f"p2{r}", bufs=2) as p2, \
                     tc.tile_pool(name=f"p2pt{r}", bufs=CFG["pt_bufs"]) as p2pt, \
                     tc.tile_pool(name=f"p2aps{r}", bufs=CFG["aT_bufs"],
                                  space="PSUM") as p2aps, \
                     tc.tile_pool(name=f"p2sps{r}", bufs=CFG["st_bufs"],
                                  space="PSUM") as p2sps, \
                     tc.tile_pool(name=f"p3{r}", bufs=4) as p3, \
                     tc.tile_pool(name=f"p3ps{r}", bufs=CFG["py_bufs"],
                                  space="PSUM") as p3ps:
                    for rg in range(NREG):
                        for p_i in range(NPAIR):
                            _emit_attention_region(
                                nc, tc, mybir, r, p_i, rg, qT_sb, kT_sb,
                                v_sb, tri_sb, ones_sb, aT_sb, p2, p2pt,
                                p2aps, p2sps, p3ps)
                        if CFG.get("py_from_st") and rg == NREG - 1:
                            _emit_out_proj_tt(nc, tc, mybir, r, rg, yT_r,
                                              aT_sb, wo_sb, p3, p2sps,
                                              py_tag="st")
                        else:
                            _emit_out_proj_tt(nc, tc, mybir, r, rg, yT_r,
                                              aT_sb, wo_sb, p3, p3ps)

    nc.compile()
    return nc


def make_core_inputs(inputs, core=0):
    """Host-side shard + bf16 cast for one core."""
    import ml_dtypes
    bf16 = ml_dtypes.bfloat16
    x = np.asarray(inputs["x"], dtype=np.float32)
    w_qkv = np.asarray(inputs["w_qkv"], dtype=np.float32)
    w_out = np.asarray(inputs["w_out"], dtype=np.float32)
    b, g = core // 2, core % 2
    sl = slice(CL * g, CL * g + CL)
    tri = np.triu(np.ones((P, P), dtype=np.float32))
    return {
        "xT": np.ascontiguousarray(x[b].T).astype(bf16),
        "wqT": np.ascontiguousarray(w_qkv[0 * D:1 * D][sl].T).astype(bf16),
        "wkT": np.ascontiguousarray(w_qkv[1 * D:2 * D][sl].T).astype(bf16),
        "wvT": np.ascontiguousarray(w_qkv[2 * D:3 * D][sl].T).astype(bf16),
        "woT": np.ascontiguousarray(w_out[:, sl].T).astype(bf16),
        "tri": tri.astype(bf16),
    }


def kernel(x, w_qkv, w_out):
    from concourse import bass_utils

    if "nc" not in _CACHE:
        _CACHE["nc"] = _build()
    nc = _CACHE["nc"]

    inputs = {"x": x, "w_qkv": w_qkv, "w_out": w_out}
    in_maps = [make_core_inputs(inputs, core=c) for c in range(8)]

    res = bass_utils.run_bass_kernel_spmd(nc, in_maps, core_ids=list(range(8)))
    outs = res.results

    y = np.empty((B, T, D), dtype=np.float32)
    for b in range(B):
        y[b] = (outs[2 * b]["yT"] + outs[2 * b + 1]["yT"]).T
    return y
